# revision 7
# baseline (speedup 1.0000x reference)
"""Trainium2 Bass kernel for nn_CanonicalCov1D (strided dual-projection covariance).

Math (reference):
  shift = W = 128, STRIDE = 8, L = T - 128 = 8064, NWIN = 993
  win1[b,n,:] = X[b, 8n : 8n+128],  win2[b,n,:] = X[b, 128+8n : 256+8n]
  proj_i = win_i @ weight_i  (per (LAT, C))
  cov[b,n,c] = mean_l[(proj1 - mean_l proj1) * (proj2 - mean_l proj2)] + bias

Key simplifications:
  * Centering projections over LAT == projecting with LAT-centered weights:
    center (and 1/LAT-scale) weights on the host, skip mean subtraction.
  * win2[n] == win_full[n+16] (shift = 16*STRIDE): one im2col serves both.
  * l-major weight layout ([w, l*64+c]) puts (l, c) on PSUM partitions, so
    the LAT-reduction becomes a constant-selector matmul that ACCUMULATES
    across all chunks in one PSUM bank — no DVE reduction pass at all.
  * Output lands [c(64 partitions), n] so the bias folds into the scalar
    engine's PSUM->SBUF exit copy (per-partition bias). Final [b, c, n] ->
    [b, n, c] permute happens on the host during unsharding.

v2 pipeline (per chunk-pair jp, 2 chunks of 128 (l,c)-rows x NT windows):
  PE:  p1 chunks into a 3-bank rotating mega tile (chunk j -> slice j%3),
       p2 chunks into a 4-bank mega tile (clean (0,1)/(2,3) pair windows).
  ACT: TWO single-chunk copies [128, NT] PSUM->SBUF f32. Singles, not a
       pair-copy: the 3-slice rotation reuses a slice one jp later, so the
       chain ACT(jp) -> p1-matmul(jp+1) -> ACT(jp+1) must stay shorter
       than the ACT busy time or the whole pipeline goes latency-bound
       (a pair-copy serialized it at 2.1us/jp; singles overlap at 1.14).
  DVE: ONE pair-mul [128, 2, NT] (SBUF f32 x PSUM f32 -> bf16, 1192ns
       vs 2x672) — DVE and ACT co-pace at ~1.2us/jp.
  GPS: pair-add bf16 -> p12sum.
  PE:  selector matmul accumulates the LAT-reduction over all 8 jp.
Startup: weight copies ride the GPSIMD software-DGE queue while the im2col
  xbar transposes ride the Sync queue — the queues run concurrently and no
  xbar mode transition (~3us) blocks the first tile. Output DMAs follow on
  the sync queue, whose single transpose->copy transition hides behind the
  first tile's compute.
"""

import numpy as np

# ---- problem constants (hardcoded; kernel.py must be self-contained) ----
B, T = 32, 8192
W, LAT, C = 128, 32, 64
STRIDE = 8
NWIN = 993            # output windows
NPAD = 1040           # winT free size (2*512 + 16)
N_CORES = 8
BPC = B // N_CORES    # batches per core
NTILES = 2            # 512-window tiles (512 + 481)
NCHUNKS = 16          # weight-column chunks of 128 (= 2 LAT values each)
NJP = NCHUNKS // 2    # chunk pairs per tile

_CACHE = {}


def _build():
    """Build the per-core Bass program."""
    import os

    # The default dependency tracker caps pairwise overlap work and falls
    # back to whole-tile edges past the cap, which serializes the sliced
    # PSUM mega-tiles below. The exhaustive check keeps edges slice-precise.
    os.environ["TILE_EXHAUSTIVE_MEMORY_SHARE_CHECK"] = "1"

    import concourse.bass as bass
    import concourse.mybir as mybir
    import concourse.tile as tile
    from concourse import bacc

    f32 = mybir.dt.float32
    bf16 = mybir.dt.bfloat16

    nc = bacc.Bacc(
        "TRN2",
        target_bir_lowering=False,
        debug=False,
        enable_asserts=False,
    )

    x_dram = nc.dram_tensor("x", [BPC, T + 256], bf16, kind="ExternalInput")
    w_dram = nc.dram_tensor("w", [W, 2 * LAT * C], bf16, kind="ExternalInput")
    sel_dram = nc.dram_tensor("sel", [W, C], bf16, kind="ExternalInput")
    bias_dram = nc.dram_tensor("bias", [C, 1], f32, kind="ExternalInput")
    out_dram = nc.dram_tensor("out", [BPC, C, NWIN], f32, kind="ExternalOutput")

    with tile.TileContext(nc) as tc:
        with (
            tc.tile_pool(name="consts", bufs=1) as consts,
            tc.tile_pool(name="wins", bufs=2) as wins,
            tc.tile_pool(name="prods", bufs=3) as prods,
            tc.tile_pool(name="outs", bufs=2) as outs,
            tc.tile_pool(name="psum", bufs=1, space="PSUM") as psum,
        ):
            # --- startup DMAs -------------------------------------------
            # weight copies on the GPSIMD software-DGE queue, transposes on
            # Sync: the queues run concurrently, so the first proj matmul
            # only waits ~1.6us for winT[0], not for an xbar transition.
            w_sb = consts.tile([W, 2 * LAT * C], bf16)
            # chunks 0-1 of both projections unblock jp0
            nc.gpsimd.dma_start(w_sb[:, 0:256], w_dram.ap()[:, 0:256])
            nc.gpsimd.dma_start(w_sb[:, 2048:2304], w_dram.ap()[:, 2048:2304])
            winTs = []
            for b in range(BPC):
                wt = wins.tile([128, NPAD], bf16, name=f"winT{b}", tag="winT", bufs=4)
                v_main = bass.AP(
                    tensor=x_dram,
                    offset=b * (T + 256),
                    ap=[[STRIDE, NPAD], [1, W]],
                )
                nc.sync.dma_start_transpose(wt[:], v_main)
                winTs.append(wt)
            # the rest of tile-0's chunks, then everything else
            nc.gpsimd.dma_start(w_sb[:, 256:1024], w_dram.ap()[:, 256:1024])
            nc.gpsimd.dma_start(w_sb[:, 2304:3072], w_dram.ap()[:, 2304:3072])
            sel_sb = consts.tile([W, C], bf16)
            nc.gpsimd.dma_start(sel_sb[:], sel_dram.ap())
            bias_sb = consts.tile([C, 1], f32)
            nc.gpsimd.dma_start(bias_sb[:], bias_dram.ap())
            nc.gpsimd.dma_start(w_sb[:, 1024:2048], w_dram.ap()[:, 1024:2048])
            nc.gpsimd.dma_start(w_sb[:, 3072:4096], w_dram.ap()[:, 3072:4096])

            # --- PSUM layout (exactly 8 banks) --------------------------
            # p1mega: 3 banks, chunk j -> slice j%3 (ACT pair-copy windows)
            # p2mega: 4 banks, chunk j -> slice pair (0,1)/(2,3)
            # selout: 1 bank, LAT-reduction accumulator
            p1mega = psum.tile([128, 3 * 512], f32, tag="p1mega", bufs=1)
            p2mega = psum.tile([128, 4 * 512], f32, tag="p2mega", bufs=1)

            # dependency-free warmup matmuls into the selout bank: keep the
            # PE busy while the startup DMA chain is in flight so the HAM
            # clock gate reaches 8/8 before the real matmuls start
            warm_sb = consts.tile([128, 64], bf16)
            nc.gpsimd.memset(warm_sb[:], 0.0)
            warm_ps = psum.tile([64, 512], f32, tag="selout")
            for i in range(8):
                nc.tensor.matmul(
                    warm_ps[:],
                    warm_sb[:],
                    warm_sb[:, None, :].to_broadcast((128, 8, 64)),
                    start=(i == 0),
                    stop=(i == 7),
                )

            p1v = p1mega.rearrange("p (s n) -> p s n", s=3)
            p2v = p2mega.rearrange("p (s n) -> p s n", s=4)

            for b in range(BPC):
                winT = winTs[b]

                for t in range(NTILES):
                    # t=1 has only 481 real windows; don't compute the pad
                    NT = 512 if t == 0 else NWIN - 512
                    selout = psum.tile([C, 512], f32, tag="selout", bufs=1)
                    rhs1 = winT[:, t * 512 : t * 512 + NT]
                    rhs2 = winT[:, t * 512 + 16 : t * 512 + 16 + NT]
                    for jp in range(NJP):
                        j0, j1 = 2 * jp, 2 * jp + 1
                        # p1 chunk matmuls into the rotating slices (j%3)
                        nc.tensor.matmul(
                            p1v[:, j0 % 3, 0:NT],
                            w_sb[:, j0 * 128 : j0 * 128 + 128],
                            rhs1,
                            start=True,
                            stop=True,
                        )
                        nc.tensor.matmul(
                            p1v[:, j1 % 3, 0:NT],
                            w_sb[:, j1 * 128 : j1 * 128 + 128],
                            rhs1,
                            start=True,
                            stop=True,
                        )
                        # p2 chunk matmuls into the clean pair window
                        q0 = 0 if jp % 2 == 0 else 2
                        nc.tensor.matmul(
                            p2v[:, q0, 0:NT],
                            w_sb[:, 2048 + j0 * 128 : 2048 + j0 * 128 + 128],
                            rhs2,
                            start=True,
                            stop=True,
                        )
                        nc.tensor.matmul(
                            p2v[:, q0 + 1, 0:NT],
                            w_sb[:, 2048 + j1 * 128 : 2048 + j1 * 128 + 128],
                            rhs2,
                            start=True,
                            stop=True,
                        )
                        # TWO single ACT copies: each unblocks its slice's
                        # next-jp matmul as soon as it lands
                        p1c = prods.tile([128, 1024], f32, tag="p1c", bufs=3)
                        p1cv = p1c.rearrange("p (a n) -> p a n", a=2)
                        nc.scalar.copy(p1cv[:, 0, 0:NT], p1v[:, j0 % 3, 0:NT])
                        nc.scalar.copy(p1cv[:, 1, 0:NT], p1v[:, j1 % 3, 0:NT])
                        # ONE pair-mul on the DVE (the pacing op)
                        p12 = prods.tile([128, 1024], bf16, tag="p12", bufs=3)
                        p12v = p12.rearrange("p (a n) -> p a n", a=2)
                        nc.vector.tensor_mul(
                            p12v[:, :, 0:NT],
                            p1cv[:, :, 0:NT],
                            p2v[:, q0 : q0 + 2, 0:NT],
                        )
                        # pre-add the chunk pair on the (otherwise idle)
                        # GPSIMD engine, halving the selector matmuls
                        p12sum = prods.tile([128, 512], bf16, tag="p12sum", bufs=3)
                        nc.gpsimd.tensor_add(
                            p12sum[:, 0:NT], p12v[:, 0, 0:NT], p12v[:, 1, 0:NT]
                        )
                        # LAT-reduction on the tensor engine: constant
                        # selector sums l-rows per c, accumulating in PSUM
                        nc.tensor.matmul(
                            selout[:, 0:NT],
                            sel_sb[:],
                            p12sum[:, 0:NT],
                            start=(jp == 0),
                            stop=(jp == NJP - 1),
                        )
                    # exit + bias in one scalar-engine op
                    st = outs.tile([C, 512], f32, tag="st")
                    nc.scalar.activation(
                        st[:, 0:NT],
                        selout[:, 0:NT],
                        mybir.ActivationFunctionType.Identity,
                        bias=bias_sb[:],
                    )
                    n0 = t * 512
                    nc.scalar.dma_start(
                        out_dram.ap()[b, :, n0 : n0 + NT], st[:, 0:NT]
                    )

    nc.compile()
    return nc


def _prep_inputs(X, weight1, weight2, bias):
    import ml_dtypes

    X = np.asarray(X, dtype=np.float32)
    weight1 = np.asarray(weight1, dtype=np.float32)
    weight2 = np.asarray(weight2, dtype=np.float32)
    bias = np.asarray(bias, dtype=np.float32)

    # center over LAT, fold 1/LAT into proj1's weights; l-major layout
    w1c = weight1 - weight1.mean(axis=1, keepdims=True)
    w2c = weight2 - weight2.mean(axis=1, keepdims=True)
    w1p = (w1c / LAT).reshape(W, LAT * C)
    w2p = w2c.reshape(W, LAT * C)
    wcat = np.concatenate([w1p, w2p], axis=1).astype(ml_dtypes.bfloat16)

    xpad = np.zeros((B, T + 256), dtype=np.float32)
    xpad[:, :T] = X
    xb = xpad.astype(ml_dtypes.bfloat16)
    sel = (np.arange(W)[:, None] % C == np.arange(C)[None, :]).astype(
        ml_dtypes.bfloat16
    )
    bias_col = np.ascontiguousarray(bias[:, None]).astype(np.float32)

    in_maps = []
    for i in range(N_CORES):
        in_maps.append(
            {
                "x": np.ascontiguousarray(xb[i * BPC : (i + 1) * BPC]),
                "w": wcat,
                "sel": sel,
                "bias": bias_col,
            }
        )
    return in_maps


def run_with_results(X, weight1, weight2, bias, trace=False, trace_cores=None):
    from concourse import bass_utils

    if "nc" not in _CACHE:
        _CACHE["nc"] = _build()
    nc = _CACHE["nc"]
    in_maps = _prep_inputs(X, weight1, weight2, bias)
    res = bass_utils.run_bass_kernel_spmd(
        nc,
        in_maps,
        core_ids=list(range(N_CORES)),
        trace=trace,
        trace_cores=trace_cores,
    )
    # results are [b, c, n]; transpose to [b, n, c] while unsharding
    out = np.concatenate(
        [res.results[i]["out"] for i in range(N_CORES)], axis=0
    ).transpose(0, 2, 1)
    return np.ascontiguousarray(out, dtype=np.float32), res


def kernel(**inputs):
    out, _ = run_with_results(
        inputs["X"], inputs["weight1"], inputs["weight2"], inputs["bias"]
    )
    return out


# revision 8
# speedup vs baseline: 1.5852x; 1.5852x over previous
"""Trainium2 Bass kernel for nn_CanonicalCov1D (strided dual-projection covariance).

Math (reference):
  shift = W = 128, STRIDE = 8, L = T - 128 = 8064, NWIN = 993
  win1[b,n,:] = X[b, 8n : 8n+128],  win2[b,n,:] = X[b, 128+8n : 256+8n]
  proj_i = win_i @ weight_i  (per (LAT, C))
  cov[b,n,c] = mean_l[(proj1 - mean_l proj1) * (proj2 - mean_l proj2)] + bias

Key simplifications:
  * Centering projections over LAT == projecting with LAT-centered weights:
    center (and 1/LAT-scale) weights on the host, skip mean subtraction.
  * win2[n] == win_full[n+16] (shift = 16*STRIDE): one im2col serves both.
  * l-major weight layout ([w, l*64+c]) puts (l, c) on PSUM partitions, so
    the LAT-reduction becomes a constant-selector matmul that ACCUMULATES
    across all 16 chunks in one PSUM bank — no DVE reduction pass at all.
  * Output lands [c(64 partitions), n] so the bias folds into the scalar
    engine's PSUM->SBUF exit copy (per-partition bias). Final [b, c, n] ->
    [b, n, c] permute happens on the host during unsharding.

Per-core device pipeline (data-parallel over batch, 4 batches/core):
  1. im2col: dma_start_transpose builds winT [128(w), 1040(n)] bf16 from
     the overlapping-window view of X. The transposes ride the Sync HWDGE
     queue while the weight copies ride the GPSIMD software-DGE queue, so
     the first tile's weights and windows land concurrently (~1.6us) with
     no xbar mode-transition (~3us) in the critical path.
  2. per (batch, 512-window tile t, chunk pair jp):
       p1pair = [W1c_j0 | W1c_j1]^T @ winT[:, t]   [128, 2x512] PSUM
       p2_j   = W2c_j^T @ winT[:, t + 16]          [128, 512] PSUM x2
       ACT: p1pair -> SBUF (one strided pair-copy)
       DVE: p12_j = p1c_j * p2_j (bf16) x2         <- pacing engine
       GPS: p12sum = p12_j0 + p12_j1
       PE:  selout += sel^T @ p12sum               [64, 512] PSUM, accum
  3. ACT: selout + bias -> SBUF, DMA out as [b, c, n] on the Scalar HWDGE.

PSUM budget (8 banks): p1pair [128,1024] x2 bufs (4) + p2 [128,512] x3 (3)
+ selout (1). Pairing BOTH sides needs 9 banks, and pair-consumers on a
3-slot rotation serialize (consumer sits in the slot-reuse ring), so p2
stays single-chunk — the DVE's 2x(120+512)cyc muls set the ~1.34us/jp pace.
"""

import numpy as np

# ---- problem constants (hardcoded; kernel.py must be self-contained) ----
B, T = 32, 8192
W, LAT, C = 128, 32, 64
STRIDE = 8
NWIN = 993            # output windows
NPAD = 1040           # winT free size (2*512 + 16)
N_CORES = 8
BPC = B // N_CORES    # batches per core
NTILES = 2            # 512-window tiles (512 + 481)
NCHUNKS = 16          # weight-column chunks of 128 (= 2 LAT values each)

_CACHE = {}


def _build():
    """Build the per-core Bass program."""
    import os

    # The default dependency tracker caps pairwise overlap work and falls
    # back to conservative whole-tile edges past the cap; exhaustive mode
    # keeps the PSUM pair-tile slices precise.
    os.environ["TILE_EXHAUSTIVE_MEMORY_SHARE_CHECK"] = "1"

    import concourse.bass as bass
    import concourse.mybir as mybir
    import concourse.tile as tile
    from concourse import bacc

    f32 = mybir.dt.float32
    bf16 = mybir.dt.bfloat16

    nc = bacc.Bacc(
        "TRN2",
        target_bir_lowering=False,
        debug=False,
        enable_asserts=False,
    )

    x_dram = nc.dram_tensor("x", [BPC, T + 256], bf16, kind="ExternalInput")
    w_dram = nc.dram_tensor("w", [W, 2 * LAT * C], bf16, kind="ExternalInput")
    sel_dram = nc.dram_tensor("sel", [W, C], bf16, kind="ExternalInput")
    bias_dram = nc.dram_tensor("bias", [C, 1], f32, kind="ExternalInput")
    out_dram = nc.dram_tensor("out", [BPC, C, NWIN], f32, kind="ExternalOutput")

    with tile.TileContext(nc) as tc:
        with (
            tc.tile_pool(name="consts", bufs=1) as consts,
            tc.tile_pool(name="wins", bufs=2) as wins,
            tc.tile_pool(name="prods", bufs=4) as prods,
            tc.tile_pool(name="outs", bufs=2) as outs,
            tc.tile_pool(name="psum", bufs=1, space="PSUM") as psum,
        ):
            # --- startup: transposes on Sync HWDGE, weights on GPSIMD DGE
            winTs = []
            for b in range(BPC):
                wt = wins.tile([128, NPAD], bf16, name=f"winT{b}", tag="winT", bufs=4)
                v_main = bass.AP(
                    tensor=x_dram,
                    offset=b * (T + 256),
                    ap=[[STRIDE, NPAD], [1, W]],
                )
                nc.sync.dma_start_transpose(wt[:], v_main)
                winTs.append(wt)
            w_sb = consts.tile([W, 2 * LAT * C], bf16)
            # chunks 0-1 of both projections unblock jp0; then the rest of
            # tile 0's chunks; then everything else
            nc.gpsimd.dma_start(w_sb[:, 0:256], w_dram.ap()[:, 0:256])
            nc.gpsimd.dma_start(w_sb[:, 2048:2304], w_dram.ap()[:, 2048:2304])
            nc.gpsimd.dma_start(w_sb[:, 256:1024], w_dram.ap()[:, 256:1024])
            nc.gpsimd.dma_start(w_sb[:, 2304:3072], w_dram.ap()[:, 2304:3072])
            sel_sb = consts.tile([W, C], bf16)
            nc.gpsimd.dma_start(sel_sb[:], sel_dram.ap())
            bias_sb = consts.tile([C, 1], f32)
            nc.gpsimd.dma_start(bias_sb[:], bias_dram.ap())
            nc.gpsimd.dma_start(w_sb[:, 1024:2048], w_dram.ap()[:, 1024:2048])
            nc.gpsimd.dma_start(w_sb[:, 3072:4096], w_dram.ap()[:, 3072:4096])

            # dependency-free warmup matmuls: run while the startup DMA
            # chain is in flight so the PE clock gate (HAM) is already at
            # full rate when the real matmuls start
            warm_sb = consts.tile([128, 64], bf16)
            nc.gpsimd.memset(warm_sb[:], 0.0)
            warm_ps = psum.tile([64, 512], f32, tag="selout")
            for i in range(8):
                nc.tensor.matmul(
                    warm_ps[:],
                    warm_sb[:],
                    warm_sb[:, None, :].to_broadcast((128, 8, 64)),
                    start=(i == 0),
                    stop=(i == 7),
                )

            for b in range(BPC):
                winT = winTs[b]

                for t in range(NTILES):
                    # t=1 has only 481 real windows; don't compute the pad
                    NT = 512 if t == 0 else NWIN - 512
                    selout = psum.tile([C, 512], f32, tag="selout", bufs=1)
                    rhs1 = winT[:, t * 512 : t * 512 + NT]
                    rhs2 = winT[:, t * 512 + 16 : t * 512 + 16 + NT]
                    for jp in range(NCHUNKS // 2):
                        # both chunks' p1 share one 2-bank PSUM tile so a
                        # single scalar-engine op exits the pair to SBUF
                        p1pair = psum.tile([128, 1024], f32, tag="p1pair", bufs=2)
                        p2s = []
                        for qi, j in enumerate((2 * jp, 2 * jp + 1)):
                            nc.tensor.matmul(
                                p1pair[:, qi * 512 : qi * 512 + NT],
                                w_sb[:, j * 128 : j * 128 + 128],
                                rhs1,
                                start=True,
                                stop=True,
                            )
                            p2 = psum.tile([128, 512], f32, tag="p2", bufs=3)
                            nc.tensor.matmul(
                                p2[:, 0:NT],
                                w_sb[:, 2048 + j * 128 : 2048 + j * 128 + 128],
                                rhs2,
                                start=True,
                                stop=True,
                            )
                            p2s.append(p2)
                        # only one PSUM operand per DVE op: stage p1 pair
                        # through SBUF on the scalar engine
                        p1c = prods.tile([128, 1024], f32, tag="p1c", bufs=4)
                        nc.scalar.copy(
                            p1c.rearrange("p (q n) -> p q n", q=2)[:, :, 0:NT],
                            p1pair.rearrange("p (q n) -> p q n", q=2)[:, :, 0:NT],
                        )
                        p12s = []
                        for qi in range(2):
                            p12 = prods.tile([128, 512], bf16, tag="p12", bufs=8)
                            nc.vector.tensor_mul(
                                p12[:, 0:NT],
                                p1c[:, qi * 512 : qi * 512 + NT],
                                p2s[qi][:, 0:NT],
                            )
                            p12s.append(p12)
                        # pre-add the chunk pair on the (otherwise idle)
                        # GPSIMD engine, halving the selector matmuls
                        p12sum = prods.tile([128, 512], bf16, tag="p12sum", bufs=4)
                        nc.gpsimd.tensor_add(
                            p12sum[:, 0:NT], p12s[0][:, 0:NT], p12s[1][:, 0:NT]
                        )
                        # LAT-reduction on the tensor engine: constant
                        # selector sums l-rows per c, accumulating in PSUM
                        nc.tensor.matmul(
                            selout[:, 0:NT],
                            sel_sb[:],
                            p12sum[:, 0:NT],
                            start=(jp == 0),
                            stop=(jp == NCHUNKS // 2 - 1),
                        )
                    # exit + bias in one scalar-engine op
                    st = outs.tile([C, 512], f32, tag="st")
                    nc.scalar.activation(
                        st[:, 0:NT],
                        selout[:, 0:NT],
                        mybir.ActivationFunctionType.Identity,
                        bias=bias_sb[:],
                    )
                    n0 = t * 512
                    nc.scalar.dma_start(
                        out_dram.ap()[b, :, n0 : n0 + NT], st[:, 0:NT]
                    )

    nc.compile()
    return nc


def _prep_inputs(X, weight1, weight2, bias):
    import ml_dtypes

    X = np.asarray(X, dtype=np.float32)
    weight1 = np.asarray(weight1, dtype=np.float32)
    weight2 = np.asarray(weight2, dtype=np.float32)
    bias = np.asarray(bias, dtype=np.float32)

    # center over LAT, fold 1/LAT into proj1's weights; l-major layout
    w1c = weight1 - weight1.mean(axis=1, keepdims=True)
    w2c = weight2 - weight2.mean(axis=1, keepdims=True)
    w1p = (w1c / LAT).reshape(W, LAT * C)
    w2p = w2c.reshape(W, LAT * C)
    wcat = np.concatenate([w1p, w2p], axis=1).astype(ml_dtypes.bfloat16)

    xpad = np.zeros((B, T + 256), dtype=np.float32)
    xpad[:, :T] = X
    xb = xpad.astype(ml_dtypes.bfloat16)
    sel = (np.arange(W)[:, None] % C == np.arange(C)[None, :]).astype(
        ml_dtypes.bfloat16
    )
    bias_col = np.ascontiguousarray(bias[:, None]).astype(np.float32)

    in_maps = []
    for i in range(N_CORES):
        in_maps.append(
            {
                "x": np.ascontiguousarray(xb[i * BPC : (i + 1) * BPC]),
                "w": wcat,
                "sel": sel,
                "bias": bias_col,
            }
        )
    return in_maps


def run_with_results(X, weight1, weight2, bias, trace=False, trace_cores=None):
    from concourse import bass_utils

    if "nc" not in _CACHE:
        _CACHE["nc"] = _build()
    nc = _CACHE["nc"]
    in_maps = _prep_inputs(X, weight1, weight2, bias)
    res = bass_utils.run_bass_kernel_spmd(
        nc,
        in_maps,
        core_ids=list(range(N_CORES)),
        trace=trace,
        trace_cores=trace_cores,
    )
    # results are [b, c, n]; transpose to [b, n, c] while unsharding
    out = np.concatenate(
        [res.results[i]["out"] for i in range(N_CORES)], axis=0
    ).transpose(0, 2, 1)
    return np.ascontiguousarray(out, dtype=np.float32), res


def kernel(**inputs):
    out, _ = run_with_results(
        inputs["X"], inputs["weight1"], inputs["weight2"], inputs["bias"]
    )
    return out


# revision 9
# speedup vs baseline: 1.7890x; 1.1285x over previous
"""Trainium2 Bass kernel for nn_CanonicalCov1D (strided dual-projection covariance).

Math (reference):
  shift = W = 128, STRIDE = 8, L = T - 128 = 8064, NWIN = 993
  win1[b,n,:] = X[b, 8n : 8n+128],  win2[b,n,:] = X[b, 128+8n : 256+8n]
  proj_i = win_i @ weight_i  (per (LAT, C))
  cov[b,n,c] = mean_l[(proj1 - mean_l proj1) * (proj2 - mean_l proj2)] + bias

Key simplifications:
  * Centering projections over LAT == projecting with LAT-centered weights:
    center (and 1/LAT-scale) weights on the host, skip mean subtraction.
  * win2[n] == win_full[n+16] (shift = 16*STRIDE): one im2col serves both.
  * l-major weight layout ([w, l*64+c]) puts (l, c) on PSUM partitions, so
    the LAT-reduction becomes a constant-selector matmul that ACCUMULATES
    across all 16 chunks in one PSUM bank — no DVE reduction pass at all.
  * Output lands [c(64 partitions), n] so the bias folds into the scalar
    engine's PSUM->SBUF exit copy (per-partition bias). Final [b, c, n] ->
    [b, n, c] permute happens on the host during unsharding.

Per-core device pipeline (data-parallel over batch, 4 batches/core):
  1. im2col: dma_start_transpose builds winT [128(w), 1040(n)] bf16 from
     the overlapping-window view of X. The transposes ride the Sync HWDGE
     queue while the weight copies ride the GPSIMD software-DGE queue, so
     the first tile's weights and windows land concurrently (~1.6us) with
     no xbar mode-transition (~3us) in the critical path.
  2. per (batch, 512-window tile t, chunk pair jp):
       p1pair = [W1c_j0 | W1c_j1]^T @ winT[:, t]   [128, 2x512] PSUM
       p2_j   = W2c_j^T @ winT[:, t + 16]          [128, 512] PSUM x2
       ACT: p1pair -> SBUF (one strided pair-copy)
       DVE: p12_j = p1c_j * p2_j (bf16) x2         <- pacing engine
       GPS: p12sum = p12_j0 + p12_j1
       PE:  selout += sel^T @ p12sum               [64, 512] PSUM, accum
  3. ACT: selout + bias -> SBUF, DMA out as [b, c, n] on the Scalar HWDGE.

PSUM budget (8 banks): p1pair [128,1024] x2 bufs (4) + p2 [128,512] x3 (3)
+ selout (1). Pairing BOTH sides needs 9 banks, and pair-consumers on a
3-slot rotation serialize (consumer sits in the slot-reuse ring), so p2
stays single-chunk — the DVE's 2x(120+512)cyc muls set the ~1.34us/jp pace.
"""

import numpy as np

# ---- problem constants (hardcoded; kernel.py must be self-contained) ----
B, T = 32, 8192
W, LAT, C = 128, 32, 64
STRIDE = 8
NWIN = 993            # output windows
NPAD = 1040           # winT free size (2*512 + 16)
N_CORES = 8
BPC = B // N_CORES    # batches per core
NTILES = 2            # 512-window tiles (512 + 481)
NCHUNKS = 16          # weight-column chunks of 128 (= 2 LAT values each)

_CACHE = {}


def _build():
    """Build the per-core Bass program."""
    import os

    # The default dependency tracker caps pairwise overlap work and falls
    # back to conservative whole-tile edges past the cap; exhaustive mode
    # keeps the PSUM pair-tile slices precise.
    os.environ["TILE_EXHAUSTIVE_MEMORY_SHARE_CHECK"] = "1"

    import concourse.bass as bass
    import concourse.mybir as mybir
    import concourse.tile as tile
    from concourse import bacc

    f32 = mybir.dt.float32
    bf16 = mybir.dt.bfloat16

    nc = bacc.Bacc(
        "TRN2",
        target_bir_lowering=False,
        debug=False,
        enable_asserts=False,
    )

    x_dram = nc.dram_tensor("x", [BPC, T + 256], bf16, kind="ExternalInput")
    w_dram = nc.dram_tensor("w", [W, 2 * LAT * C], bf16, kind="ExternalInput")
    sel_dram = nc.dram_tensor("sel", [W, C], bf16, kind="ExternalInput")
    bias_dram = nc.dram_tensor("bias", [C, 1], f32, kind="ExternalInput")
    out_dram = nc.dram_tensor("out", [BPC, C, NWIN], f32, kind="ExternalOutput")

    with tile.TileContext(nc) as tc:
        with (
            tc.tile_pool(name="consts", bufs=1) as consts,
            tc.tile_pool(name="wins", bufs=2) as wins,
            tc.tile_pool(name="prods", bufs=4) as prods,
            tc.tile_pool(name="outs", bufs=2) as outs,
            tc.tile_pool(name="psum", bufs=1, space="PSUM") as psum,
        ):
            # --- startup DMAs, all on the Sync queue: DMAs serialize
            # globally on xbar mode transitions (~3.4us each), so group
            # them — one copy group, one transpose group, one copy group.
            w_sb = consts.tile([W, 2 * LAT * C], bf16)
            # quarters 0 (proj1 j<8) and 2 (proj2 j<8) feed the first chunks
            nc.sync.dma_start(w_sb[:, 0:1024], w_dram.ap()[:, 0:1024])
            nc.sync.dma_start(w_sb[:, 2048:3072], w_dram.ap()[:, 2048:3072])
            winTs = []
            for b in range(BPC):
                wt = wins.tile([128, NPAD], bf16, name=f"winT{b}", tag="winT", bufs=4)
                v_main = bass.AP(
                    tensor=x_dram,
                    offset=b * (T + 256),
                    ap=[[STRIDE, NPAD], [1, W]],
                )
                nc.sync.dma_start_transpose(wt[:], v_main)
                winTs.append(wt)
            for wq in (1, 3):
                nc.sync.dma_start(
                    w_sb[:, wq * 1024 : wq * 1024 + 1024],
                    w_dram.ap()[:, wq * 1024 : wq * 1024 + 1024],
                )
            sel_sb = consts.tile([W, C], bf16)
            nc.sync.dma_start(sel_sb[:], sel_dram.ap())
            bias_sb = consts.tile([C, 1], f32)
            nc.sync.dma_start(bias_sb[:], bias_dram.ap())

            # dependency-free warmup matmuls: run while the startup DMA
            # chain is in flight so the PE clock gate (HAM) is already at
            # full rate when the real matmuls start
            warm_sb = consts.tile([128, 64], bf16)
            nc.gpsimd.memset(warm_sb[:], 0.0)
            warm_ps = psum.tile([64, 512], f32, tag="selout")
            for i in range(20):
                nc.tensor.matmul(
                    warm_ps[:],
                    warm_sb[:],
                    warm_sb[:, None, :].to_broadcast((128, 8, 64)),
                    start=(i == 0),
                    stop=(i == 19),
                )

            for b in range(BPC):
                winT = winTs[b]

                for t in range(NTILES):
                    # t=1 has only 481 real windows; don't compute the pad
                    NT = 512 if t == 0 else NWIN - 512
                    selout = psum.tile([C, 512], f32, tag="selout", bufs=1)
                    rhs1 = winT[:, t * 512 : t * 512 + NT]
                    rhs2 = winT[:, t * 512 + 16 : t * 512 + 16 + NT]
                    for jp in range(NCHUNKS // 2):
                        # both chunks' p1 share one 2-bank PSUM tile so a
                        # single scalar-engine op exits the pair to SBUF
                        p1pair = psum.tile([128, 1024], f32, tag="p1pair", bufs=2)
                        p2s = []
                        for qi, j in enumerate((2 * jp, 2 * jp + 1)):
                            nc.tensor.matmul(
                                p1pair[:, qi * 512 : qi * 512 + NT],
                                w_sb[:, j * 128 : j * 128 + 128],
                                rhs1,
                                start=True,
                                stop=True,
                            )
                            p2 = psum.tile([128, 512], f32, tag="p2", bufs=3)
                            nc.tensor.matmul(
                                p2[:, 0:NT],
                                w_sb[:, 2048 + j * 128 : 2048 + j * 128 + 128],
                                rhs2,
                                start=True,
                                stop=True,
                            )
                            p2s.append(p2)
                        # only one PSUM operand per DVE op: stage p1 pair
                        # through SBUF on the scalar engine
                        p1c = prods.tile([128, 1024], f32, tag="p1c", bufs=4)
                        nc.scalar.copy(
                            p1c.rearrange("p (q n) -> p q n", q=2)[:, :, 0:NT],
                            p1pair.rearrange("p (q n) -> p q n", q=2)[:, :, 0:NT],
                        )
                        p12s = []
                        for qi in range(2):
                            p12 = prods.tile([128, 512], bf16, tag="p12", bufs=8)
                            nc.vector.tensor_mul(
                                p12[:, 0:NT],
                                p1c[:, qi * 512 : qi * 512 + NT],
                                p2s[qi][:, 0:NT],
                            )
                            p12s.append(p12)
                        # pre-add the chunk pair on the (otherwise idle)
                        # GPSIMD engine, halving the selector matmuls
                        p12sum = prods.tile([128, 512], bf16, tag="p12sum", bufs=4)
                        nc.gpsimd.tensor_add(
                            p12sum[:, 0:NT], p12s[0][:, 0:NT], p12s[1][:, 0:NT]
                        )
                        # LAT-reduction on the tensor engine: constant
                        # selector sums l-rows per c, accumulating in PSUM
                        nc.tensor.matmul(
                            selout[:, 0:NT],
                            sel_sb[:],
                            p12sum[:, 0:NT],
                            start=(jp == 0),
                            stop=(jp == NCHUNKS // 2 - 1),
                        )
                    # exit + bias in one scalar-engine op
                    st = outs.tile([C, 512], f32, tag="st")
                    nc.scalar.activation(
                        st[:, 0:NT],
                        selout[:, 0:NT],
                        mybir.ActivationFunctionType.Identity,
                        bias=bias_sb[:],
                    )
                    n0 = t * 512
                    nc.scalar.dma_start(
                        out_dram.ap()[b, :, n0 : n0 + NT], st[:, 0:NT]
                    )

    nc.compile()
    return nc


def _prep_inputs(X, weight1, weight2, bias):
    import ml_dtypes

    X = np.asarray(X, dtype=np.float32)
    weight1 = np.asarray(weight1, dtype=np.float32)
    weight2 = np.asarray(weight2, dtype=np.float32)
    bias = np.asarray(bias, dtype=np.float32)

    # center over LAT, fold 1/LAT into proj1's weights; l-major layout
    w1c = weight1 - weight1.mean(axis=1, keepdims=True)
    w2c = weight2 - weight2.mean(axis=1, keepdims=True)
    w1p = (w1c / LAT).reshape(W, LAT * C)
    w2p = w2c.reshape(W, LAT * C)
    wcat = np.concatenate([w1p, w2p], axis=1).astype(ml_dtypes.bfloat16)

    xpad = np.zeros((B, T + 256), dtype=np.float32)
    xpad[:, :T] = X
    xb = xpad.astype(ml_dtypes.bfloat16)
    sel = (np.arange(W)[:, None] % C == np.arange(C)[None, :]).astype(
        ml_dtypes.bfloat16
    )
    bias_col = np.ascontiguousarray(bias[:, None]).astype(np.float32)

    in_maps = []
    for i in range(N_CORES):
        in_maps.append(
            {
                "x": np.ascontiguousarray(xb[i * BPC : (i + 1) * BPC]),
                "w": wcat,
                "sel": sel,
                "bias": bias_col,
            }
        )
    return in_maps


def run_with_results(X, weight1, weight2, bias, trace=False, trace_cores=None):
    from concourse import bass_utils

    if "nc" not in _CACHE:
        _CACHE["nc"] = _build()
    nc = _CACHE["nc"]
    in_maps = _prep_inputs(X, weight1, weight2, bias)
    res = bass_utils.run_bass_kernel_spmd(
        nc,
        in_maps,
        core_ids=list(range(N_CORES)),
        trace=trace,
        trace_cores=trace_cores,
    )
    # results are [b, c, n]; transpose to [b, n, c] while unsharding
    out = np.concatenate(
        [res.results[i]["out"] for i in range(N_CORES)], axis=0
    ).transpose(0, 2, 1)
    return np.ascontiguousarray(out, dtype=np.float32), res


def kernel(**inputs):
    out, _ = run_with_results(
        inputs["X"], inputs["weight1"], inputs["weight2"], inputs["bias"]
    )
    return out
